# revision 8
# baseline (speedup 1.0000x reference)
"""ConvCRF message-passing kernel for Trainium2 (Bass/Tile), 8-core data parallel.

Problem: B=16,C=1,H=512,W=512 images. Per image:
  K = softmax_k(exp(-0.5*(t0*dx^2+t1*dy^2+t2*(255*dI_k)^2)) * valid_k)  (9 3x3 offsets)
  pred_{t+1} = 0.5*u + 0.5*w*sum_k K_k * shift_k(pred),  10 iters, zero-pad shifts.

Sharding: batch across 8 cores (2 images/core), SPMD, no collectives.

On-chip layout (per image): flat [128 partitions, 2048 free]; partition p holds
image rows 4p..4p+3 (512 wide each). A stencil shift by (dx,dy) = flat offset
512*dx+dy = free-dim offset, except rows crossing partition boundaries.
Partition-crossing handled by full "U/D-shifted" copies built with PE shift
matmuls (identity / sub- / super-diagonal lhsT) + ScalarE PSUM->SBUF copies.
Out-of-range contributions are exactly zeroed through the kernel planes
(borders memset to 0), so wrap-around garbage reads are multiplied by 0.

Softmax denominator uses the mirror identity E_{-k}[x] = E_k[x - o_k] and the
"excess" form Etil = exp(entry)-1 so out-of-range positions contribute 0:
  S = (8+e) + sum_{k in 8 nbrs} Etil_k ;  Kfin_k = 0.5*w*(Etil_k+1)/S.
1/S via ScalarE exp(-ln(S)) (Ln+Exp share one ACT table set).
"""
import sys

if "/opt/trn_rl_repo" not in sys.path:
    sys.path.insert(0, "/opt/trn_rl_repo")

import math
import numpy as np

import concourse.bass as bass
from concourse import bacc
from concourse import mybir
from concourse import bass_utils
from concourse.tile import TileContext

B, H, W = 16, 512, 512
NCORES = 8
BPC = B // NCORES  # images per core
P = 128
R = H // P  # 4 rows per partition
F = R * W  # 2048 flat elements per partition
PAD = 8
FT = F + 2 * PAD
DT = mybir.dt.float32

B4 = [(-1, -1), (-1, 0), (-1, 1), (0, -1)]  # computed kernel planes
ALL8 = [(-1, -1), (-1, 0), (-1, 1), (0, -1), (0, 1), (1, -1), (1, 0), (1, 1)]

_cache = {}


def _shift_mats():
    ident = np.eye(P, dtype=np.float32)
    s_dn = np.eye(P, k=-1, dtype=np.float32)  # lhsT for out[m] = rhs[m+1]
    s_up = np.eye(P, k=1, dtype=np.float32)  # lhsT for out[m] = rhs[m-1]
    return np.stack([ident, s_up, s_dn])


def _build(t0, t1, t2, w):
    """Build the Bass program. Scalars are baked in as immediates."""
    c = 0.5 * t2 * 255.0 * 255.0
    nc = bacc.Bacc("TRN2", num_devices=NCORES)
    img_h = nc.declare_dram_parameter("image", [BPC, H, W], DT, isOutput=False)
    un_h = nc.declare_dram_parameter("unary", [BPC, H, W], DT, isOutput=False)
    sm_h = nc.declare_dram_parameter("shmats", [3, P, P], DT, isOutput=False)
    out_h = nc.declare_dram_parameter("out", [BPC, H, W], DT, isOutput=True)

    AF = mybir.ActivationFunctionType
    OP = mybir.AluOpType

    with TileContext(nc) as tc:
        with tc.tile_pool(name="main", bufs=1) as main, \
             tc.tile_pool(name="psum", bufs=1, space="PSUM") as psp:
            # --- constant lhsT matrices ---
            ident_t = main.tile([P, P], DT, tag="ident", name="ident")
            sup_t = main.tile([P, P], DT, tag="sup", name="sup")
            sdn_t = main.tile([P, P], DT, tag="sdn", name="sdn")
            sm_ap = sm_h.ap()
            nc.sync.dma_start(out=ident_t, in_=sm_ap[0])
            nc.sync.dma_start(out=sup_t, in_=sm_ap[1])
            nc.sync.dma_start(out=sdn_t, in_=sm_ap[2])

            def big(tag):
                return main.tile([P, FT], DT, tag=tag, name=tag)

            # persistent tiles
            pred = big("pred")
            predU = big("predU")
            predD = big("predD")
            halfu = big("halfu")
            acc = big("acc")
            tmp = big("tmp")
            rcp = big("rcp")
            img = big("img")
            sc = [big(f"sc{i}") for i in range(4)]  # rotating scratch
            kfin = {k: big(f"kfin{i}") for i, k in enumerate(ALL8 + [(0, 0)])}
            etil = {k: big(f"etil{i}") for i, k in enumerate(B4)}

            pu_ps = psp.tile([P, F], DT, tag="pu", name="pu")
            pd_ps = psp.tile([P, F], DT, tag="pd", name="pd")

            const_cols = {}

            def ccol(val):
                v = float(val)
                if v not in const_cols:
                    nm = f"c{len(const_cols)}"
                    t = main.tile([P, 1], DT, tag=nm, name=nm)
                    nc.gpsimd.memset(t, v)
                    const_cols[v] = t
                return const_cols[v]

            def data(t, off=0):
                return t[:, PAD + off:PAD + F + off]

            def chunk(t, r, off=0):
                return t[:, PAD + r * W + off:PAD + (r + 1) * W + off]

            # zero pad columns of tiles whose pads are ever read
            for t in [img, pred, predU, predD] + sc + list(etil.values()):
                nc.gpsimd.memset(t[:, 0:PAD], 0.0)
                nc.gpsimd.memset(t[:, PAD + F:FT], 0.0)

            def pe_dshift(ps, src):
                # ps[p, r, w] = src[p, r+1, w] (r<3);  ps[p, 3, w] = src[p+1, 0, w]
                for r in range(R - 1):
                    nc.tensor.matmul(ps[:, r * W:(r + 1) * W], ident_t,
                                     chunk(src, r + 1), start=True, stop=True)
                nc.tensor.matmul(ps[:, (R - 1) * W:R * W], sdn_t,
                                 chunk(src, 0), start=True, stop=True)

            def pe_ushift(ps, src):
                # ps[p, r, w] = src[p, r-1, w] (r>0);  ps[p, 0, w] = src[p-1, 3, w]
                for r in range(1, R):
                    nc.tensor.matmul(ps[:, r * W:(r + 1) * W], ident_t,
                                     chunk(src, r - 1), start=True, stop=True)
                nc.tensor.matmul(ps[:, 0:W], sup_t,
                                 chunk(src, R - 1), start=True, stop=True)

            def zero_borders(t, dx, dy, rows=False):
                # Row-border zeroing is only needed on Etil planes (softmax
                # correctness). For kfin planes the row-out-of-range pred
                # reads are already exactly 0 (PE shift-matrix zero column),
                # so kfin row borders multiply 0 and can stay unzeroed.
                # (Partition-127 starts are also illegal on compute engines.)
                if rows and dx == -1:  # row 0 = partition 0, r-block 0
                    nc.vector.memset(t[0:1, PAD:PAD + W], 0.0)
                t3 = data(t).rearrange("p (r w) -> p r w", w=W)
                if dy == -1:
                    nc.gpsimd.memset(t3[:, :, 0:1], 0.0)
                if dy == 1:
                    nc.gpsimd.memset(t3[:, :, W - 1:W], 0.0)

            def etil_ap(dx, dy, st):
                """AP reading Etil_(dx,dy) at alignment of x (output pixel)."""
                if (dx, dy) in B4:
                    return data(etil[(dx, dy)])
                if dx == 0:  # (0,1): mirror of (0,-1), read at +1
                    return data(etil[(0, -1)], 1)
                # dx == 1: mirror of (-1,-dy): staged D-shift read at dy
                return data(st[(-1, -dy)], dy)

            for b in range(BPC):
                img_dram = img_h.ap()[b].rearrange("(p r) w -> p (r w)", r=R)
                un_dram = un_h.ap()[b].rearrange("(p r) w -> p (r w)", r=R)
                out_dram = out_h.ap()[b].rearrange("(p r) w -> p (r w)", r=R)

                nc.sync.dma_start(out=data(img), in_=img_dram)
                nc.sync.dma_start(out=data(pred), in_=un_dram)
                nc.vector.tensor_scalar_mul(data(halfu), data(pred), 0.5)

                # imgU/imgD shifted copies (sc0/sc1)
                imgU, imgD, A = sc[0], sc[1], sc[2]
                pe_ushift(pu_ps, img)
                pe_dshift(pd_ps, img)
                nc.scalar.copy(data(imgU), pu_ps[:, :])
                nc.scalar.copy(data(imgD), pd_ps[:, :])

                # 4 computed Etil planes
                for (dx, dy) in B4:
                    lna = -0.5 * (t0 * dx * dx + t1 * dy * dy)
                    src = {0: img, -1: imgU, 1: imgD}[dx]
                    nc.vector.tensor_tensor(
                        out=data(A), in0=data(src, dy), in1=data(img),
                        op=OP.subtract)
                    nc.scalar.activation(data(A), data(A), AF.Square)
                    nc.scalar.activation(data(A), data(A), AF.Exp,
                                         bias=ccol(lna), scale=-c)
                    nc.scalar.activation(data(A), data(A), AF.Exp)
                    nc.vector.tensor_scalar_add(data(etil[(dx, dy)]), data(A),
                                                -1.0)
                    zero_borders(etil[(dx, dy)], dx, dy)

                # staged D-shifts of the dx=-1 planes (for mirror reads)
                st = {}
                for i, k in enumerate([(-1, -1), (-1, 0), (-1, 1)]):
                    stt = sc[i]
                    pe_dshift(pd_ps, etil[k])
                    nc.scalar.copy(data(stt), pd_ps[:, :])
                    st[k] = stt

                # S' = sum of 8 Etil reads; S = S' + (8+e)
                nc.vector.tensor_tensor(out=data(acc),
                                        in0=etil_ap(*ALL8[0], st),
                                        in1=etil_ap(*ALL8[1], st), op=OP.add)
                for k in ALL8[2:]:
                    nc.vector.tensor_tensor(out=data(acc), in0=data(acc),
                                            in1=etil_ap(*k, st), op=OP.add)
                # rcp = 0.5*w/S = exp(-ln(S'+(8+e)) + ln(0.5 w))
                nc.scalar.activation(data(acc), data(acc), AF.Ln,
                                     bias=ccol(8.0 + math.e), scale=1.0)
                nc.scalar.activation(data(rcp), data(acc), AF.Exp,
                                     bias=ccol(math.log(0.5 * w)), scale=-1.0)

                # Kfin planes
                for k in ALL8:
                    nc.vector.scalar_tensor_tensor(
                        out=data(kfin[k]), in0=etil_ap(*k, st), scalar=1.0,
                        in1=data(rcp), op0=OP.add, op1=OP.mult)
                    zero_borders(kfin[k], *k)
                nc.vector.tensor_scalar_mul(data(kfin[(0, 0)]), data(rcp),
                                            math.e)

                # --- message passing ---
                for it in range(10):
                    pe_ushift(pu_ps, pred)
                    pe_dshift(pd_ps, pred)
                    nc.scalar.copy(data(predU), pu_ps[:, :])
                    nc.scalar.copy(data(predD), pd_ps[:, :])
                    nc.vector.tensor_tensor(out=data(acc),
                                            in0=data(kfin[(0, 0)]),
                                            in1=data(pred), op=OP.mult)
                    for (dx, dy) in ALL8:
                        src = {0: pred, -1: predU, 1: predD}[dx]
                        nc.vector.tensor_tensor(out=data(tmp),
                                                in0=data(kfin[(dx, dy)]),
                                                in1=data(src, dy), op=OP.mult)
                        nc.vector.tensor_tensor(out=data(acc), in0=data(acc),
                                                in1=data(tmp), op=OP.add)
                    nc.vector.tensor_tensor(out=data(pred), in0=data(halfu),
                                            in1=data(acc), op=OP.add)

                nc.sync.dma_start(out=out_dram, in_=data(pred))
    nc.finalize()
    return nc


def _get_nc(t0, t1, t2, w):
    key = (t0, t1, t2, w)
    if key not in _cache:
        _cache[key] = _build(t0, t1, t2, w)
    return _cache[key]


def kernel(image, unary, theta, weight):
    image = np.ascontiguousarray(np.asarray(image, dtype=np.float32))
    unary = np.ascontiguousarray(np.asarray(unary, dtype=np.float32))
    t0, t1, t2 = [float(x) for x in np.asarray(theta).reshape(3)]
    w = float(np.asarray(weight).reshape(1)[0])
    nc = _get_nc(t0, t1, t2, w)
    sm = _shift_mats()
    in_maps = []
    for i in range(NCORES):
        in_maps.append({
            "image": np.ascontiguousarray(image[i * BPC:(i + 1) * BPC, 0]),
            "unary": np.ascontiguousarray(unary[i * BPC:(i + 1) * BPC, 0]),
            "shmats": sm,
        })
    res = bass_utils.run_bass_kernel_spmd(nc, in_maps, core_ids=list(range(NCORES)))
    kernel.last_results = res
    out = np.concatenate([r["out"] for r in res.results], axis=0)
    return out.reshape(B, 1, H, W).astype(np.float32)
